# revision 19
# baseline (speedup 1.0000x reference)
"""Trainium2 Bass kernel for the expert-selective time-varying SSM.

Math restructuring (vs the reference scan):
  - Track z_t = S @ x_t instead of x_t: the triangular solve disappears
    (z at step t+1 equals the previous step's z_next), x_t = S^-1 z_t is
    recovered by one batched matmul at the end.
  - K22 is zeroed before normalization, so y_t = sum_m pi_m K21_m z_t
    depends only on z_t -> computed outside the scan in batched matmuls.
  - The gate (pi) and the input drive c_t = sum_m pi_m K12_m (g u_t)
    depend only on u -> precomputed in parallel.
  - The only sequential work left: z_{t+1} = sum_m K11_m (pi_m * z_t) + c_t.

Sharding: data-parallel over B across 8 cores (B_loc = 8 per core);
small params replicated. No collectives needed.
"""

import os
import sys

for _p in ("/opt/trn_rl_repo", "/root/.axon_site/_ro/trn_rl_repo"):
    if os.path.isdir(_p) and _p not in sys.path:
        sys.path.insert(0, _p)

import numpy as np
import ml_dtypes

import concourse.bass as bass
import concourse.bacc as bacc
import concourse.tile as tile
from concourse import mybir
from concourse.bass_utils import run_bass_kernel_spmd

F32 = mybir.dt.float32
BF16 = mybir.dt.bfloat16
AF = mybir.ActivationFunctionType
ALU = mybir.AluOpType

B, T, DS, DU, DY, M, GH = 64, 1024, 128, 64, 64, 8, 64
S_DIAG_EPS = 1e-3
NCORES = 8
BL = B // NCORES          # batch per core = 8
W = 64                    # scan window (timesteps)
GELU_MODE = "hw"          # "hw": ACT Gelu; "sim": sigmoid approx (CoreSim dev)
NW = T // W               # number of windows
NTB = T * BL              # (t, b) pairs per core = 8192
CHUNK = 512               # free-dim chunk for phase matmuls
NCHUNK = NTB // CHUNK     # 16


def _np_bf16(x):
    return np.ascontiguousarray(x).astype(ml_dtypes.bfloat16)


def prepare_params(K_raw, log_gamma, S_raw, gw1, gb1, gw2, gb2):
    """Host-side parameter prep (small, O(params) only)."""
    K = np.array(K_raw, dtype=np.float64).copy()
    K[:, DS:DS + DY, DS:DS + DU] = 0.0
    norms = np.array([np.linalg.norm(K[m], 2) for m in range(M)])
    scale = np.maximum(norms, 1.0)
    K = K / scale[:, None, None]

    g = float(np.exp(np.float64(log_gamma)))
    K11 = K[:, :DS, :DS]                      # (M, DS, DS)
    K12 = K[:, :DS, DS:DS + DU] * g           # (M, DS, DU), gamma folded in
    K21 = K[:, DS:DS + DY, :DS]               # (M, DY, DS)

    S = np.tril(np.array(S_raw, dtype=np.float64))
    d = np.diagonal(S).copy()
    sp = np.where(d > 30, d, np.log1p(np.exp(np.minimum(d, 30.0)))) + S_DIAG_EPS
    np.fill_diagonal(S, sp)
    Sinv = np.linalg.inv(S)

    p = {}
    # lhsT layouts: stationary operand of matmul(out, lhsT, rhs) is [K, M_out]
    p["k11t"] = _np_bf16(K11.transpose(2, 0, 1))          # (DS_in, M, DS_out)
    p["k21t"] = _np_bf16(K21.transpose(2, 0, 1))          # (DS, M, DY)
    p["k12t"] = _np_bf16(K12.transpose(2, 0, 1))          # (DU, M, DS)
    p["gw1t"] = _np_bf16(np.array(gw1, np.float64).T)     # (DU, GH)
    gw2a = np.concatenate([np.array(gw2, np.float64).T,
                           np.array(gb2, np.float64)[None, :]], axis=0)
    p["gw2a"] = _np_bf16(gw2a)                            # (GH+1, M)
    p["gb1c"] = np.array(gb1, np.float64).reshape(GH, 1).astype(np.float32)
    p["sinvt"] = np.ascontiguousarray(Sinv.T).astype(np.float32)  # (DS, DS)
    p["idbf"] = _np_bf16(np.eye(DS))                              # c-inject
    return p


def make_in_maps(u, p):
    u = np.asarray(u, dtype=np.float32)
    in_maps = []
    for i in range(NCORES):
        m = dict(p)
        ut = np.ascontiguousarray(u[i * BL:(i + 1) * BL].transpose(2, 1, 0))
        m["ut"] = ut                       # (DU, T, BL) f32
        m["utb"] = _np_bf16(ut)
        in_maps.append(m)
    return in_maps


def build_program(nc):
    """Emit the SPMD per-core program. Returns nothing; tensors are
    declared on nc by name."""
    ut = nc.declare_dram_parameter("ut", [DU, T, BL], F32, isOutput=False)
    utb_in = nc.declare_dram_parameter("utb", [DU, T, BL], BF16, isOutput=False)
    idbf = nc.declare_dram_parameter("idbf", [DS, DS], BF16, isOutput=False)
    k11t = nc.declare_dram_parameter("k11t", [DS, M, DS], BF16, isOutput=False)
    k21t = nc.declare_dram_parameter("k21t", [DS, M, DY], BF16, isOutput=False)
    k12t = nc.declare_dram_parameter("k12t", [DU, M, DS], BF16, isOutput=False)
    gw1t = nc.declare_dram_parameter("gw1t", [DU, GH], BF16, isOutput=False)
    gw2a = nc.declare_dram_parameter("gw2a", [GH + 1, M], BF16, isOutput=False)
    gb1c = nc.declare_dram_parameter("gb1c", [GH, 1], F32, isOutput=False)
    sinvt = nc.declare_dram_parameter("sinvt", [DS, DS], F32, isOutput=False)

    y_o = nc.declare_dram_parameter("y_o", [DY, T, BL], F32, isOutput=True)
    x_o = nc.declare_dram_parameter("x_o", [DS, T, BL], F32, isOutput=True)

    pid = nc.dram_tensor("pid", [T, BL, M], BF16)  # pi scratch, (t, b, m)

    with tile.TileContext(nc) as tc:
        with (
            tc.tile_pool(name="wts", bufs=1) as wts,
            tc.tile_pool(name="big", bufs=1) as big,
            tc.tile_pool(name="ph1", bufs=2) as ph1,
            tc.tile_pool(name="winp", bufs=3) as winp,
            tc.tile_pool(name="ztwp", bufs=2) as ztwp,
            tc.tile_pool(name="ub", bufs=2) as ubp,
            tc.tile_pool(name="cw", bufs=2) as cwp,
            tc.tile_pool(name="st", bufs=2) as stp,
            tc.tile_pool(name="ps_s", bufs=3, space="PSUM") as ps_s,
            tc.tile_pool(name="ps_b", bufs=4, space="PSUM") as ps_b,
        ):
            # ---- weight preload ----
            k11s = wts.tile([DS, M, DS], BF16)
            k21s = wts.tile([DS, M, DY], BF16)
            k12s = wts.tile([DU, M, DS], BF16)
            gw1s = wts.tile([DU, GH], BF16)
            gw2s = wts.tile([GH + 1, M], BF16)
            gb1s = wts.tile([GH, 1], F32)
            sis = wts.tile([DS, DS], F32)
            ids = wts.tile([DS, DS], BF16)
            nc.sync.dma_start(k11s[:], k11t[:])
            nc.sync.dma_start(k21s[:], k21t[:])
            nc.sync.dma_start(k12s[:], k12t[:])
            nc.sync.dma_start(gw1s[:], gw1t[:])
            nc.sync.dma_start(gw2s[:], gw2a[:])
            nc.sync.dma_start(gb1s[:], gb1c[:])
            nc.sync.dma_start(sis[:], sinvt[:])
            nc.sync.dma_start(ids[:], idbf[:])

            # ---- persistent big tiles ----
            utf = big.tile([DU, NTB], F32)      # u^T, free = t*BL + b
            utb = big.tile([DU, NTB], BF16)
            zh = big.tile([DS, T * BL], F32)    # z history, slot t = z_t
            nc.sync.dma_start(utf[:], ut[:].rearrange("d t b -> d (t b)"))
            nc.sync.dma_start(utb[:], utb_in[:].rearrange("d t b -> d (t b)"))

            # ---- phase 1: gate -> pi -> DRAM (t, b, m) ----
            hh = big.tile([GH + 1, NTB], BF16)
            nc.vector.memset(hh[GH:GH + 1, :], 1.0)
            for i in range(NCHUNK):
                ph = ps_b.tile([GH, CHUNK], F32, tag="psb")
                nc.tensor.matmul(ph[:], gw1s[:], utb[:, bass.ts(i, CHUNK)],
                                 start=True, stop=True)
                if GELU_MODE == "hw":
                    nc.scalar.activation(hh[:GH, bass.ts(i, CHUNK)], ph[:],
                                         AF.Gelu, bias=gb1s[:], scale=1.0)
                else:
                    pre = ph1.tile([GH, CHUNK], F32, tag="gpre")
                    sg = ph1.tile([GH, CHUNK], F32, tag="gsg")
                    nc.scalar.activation(pre[:], ph[:], AF.Identity,
                                         bias=gb1s[:], scale=1.0)
                    nc.scalar.activation(sg[:], pre[:], AF.Sigmoid, scale=1.702)
                    nc.vector.tensor_mul(hh[:GH, bass.ts(i, CHUNK)],
                                         pre[:], sg[:])

            pl = ps_b.tile([128, NTB // 128 * M], F32, tag="psb")  # (128, 512)
            for c in range(NTB // 128):
                nc.tensor.matmul(pl[:, bass.ts(c, M)], hh[:, bass.ts(c, 128)],
                                 gw2s[:], start=True, stop=True)
            nbt = NTB // 128  # 64
            ex = ph1.tile([128, nbt * M], F32, tag="ex")
            mx = ph1.tile([128, nbt], F32, tag="mx")
            sm = ph1.tile([128, nbt], F32, tag="mx")
            pib = ph1.tile([128, nbt * M], BF16, tag="pib")
            pl3 = pl[:].rearrange("p (c m) -> p c m", m=M)
            nc.vector.tensor_reduce(mx[:], pl3, mybir.AxisListType.X, ALU.max)
            nc.vector.tensor_tensor(ex[:].rearrange("p (c m) -> p c m", m=M),
                                    pl3,
                                    mx[:, :, None].broadcast_to([128, nbt, M]),
                                    ALU.subtract)
            nc.scalar.activation(ex[:], ex[:], AF.Exp)
            nc.vector.tensor_reduce(sm[:], ex[:].rearrange("p (c m) -> p c m", m=M),
                                    mybir.AxisListType.X, ALU.add)
            nc.vector.reciprocal(sm[:], sm[:])
            nc.vector.tensor_tensor(pib[:].rearrange("p (c m) -> p c m", m=M),
                                    ex[:].rearrange("p (c m) -> p c m", m=M),
                                    sm[:, :, None].broadcast_to([128, nbt, M]),
                                    ALU.mult)
            # scatter pi -> DRAM (t, b, m); partition p=q*BL+b maps to
            # t = c*16 + q (bt = c*128 + p, bt = t*BL + b)
            NQ = 128 // BL
            for q in range(NQ):
                src = pib[q * BL:(q + 1) * BL, :].rearrange(
                    "b (c m) -> b c m", m=M)
                dst = pid[:].rearrange("(c q) b m -> q b c m", q=NQ)[q]
                nc.sync.dma_start(dst, src)

            # ---- phase 2: software-pipelined windowed scan ----
            # Per scan step t (window w, tr=t%W):
            #   PE : Id@c_t (start) + 8 accumulating K11 matmuls -> ps = z_{t+1}
            #   DVE: ztilde_{t+1} = ps (bcast over m) * P_{t+1}
            #   ACT: zh[t+1] = copy(ps)  (off critical path)
            # Window jobs (c-build for w+1, y/x for w-1, P prefetch w+2) are
            # interleaved one per scan step to fill engine idle time.
            nc.vector.memset(zh[:, 0:BL], 0.0)

            pw_tiles = {}
            ztw_tiles = {}
            pc_ps = {}
            py_ps = {}
            px_ps = {}

            def emit_pw_dma(w):
                pw = winp.tile([128, W * BL * M], BF16, tag="pw")
                nc.sync.dma_start(
                    pw[:],
                    pid[bass.ts(w, W), :, :].rearrange("t b m -> (t b m)")
                    .partition_broadcast(128)
                    .rearrange("p f -> p f"))
                pw_tiles[w] = pw

            def emit_cbuild(w, m):
                # one expert of the c-window accumulation
                if m == 0:
                    pc_ps[w] = ps_b.tile([DS, W * BL], F32, tag="psb", name="pc")
                pw = pw_tiles[w]
                ub = ubp.tile([DU, W * BL], BF16, tag="ub")
                nc.vector.tensor_tensor(
                    ub[:].rearrange("p (t b) -> p t b", b=BL),
                    utf[:, bass.ts(w, W * BL)].rearrange("p (t b) -> p t b", b=BL),
                    pw[:DU, :].rearrange("p (t b m) -> p t b m", b=BL, m=M)[:, :, :, m],
                    ALU.mult)
                nc.tensor.matmul(pc_ps[w][:], k12s[:, m, :], ub[:],
                                 start=(m == 0), stop=(m == M - 1))

            def emit_cw_copy(w):
                cw = cwp.tile([DS, W * BL], BF16, tag="cw")
                nc.scalar.copy(cw[:], pc_ps[w][:])
                cw_tiles[w] = cw

            def emit_y_mm(w, m):
                if m == 0:
                    py_ps[w] = ps_b.tile([DY, W * BL], F32, tag="psb", name="py")
                ztw4 = ztw_tiles[w][:].rearrange(
                    "p (t b m) -> p m t b", b=BL, m=M)
                nc.tensor.matmul(py_ps[w][:], k21s[:, m, :], ztw4[:, m, :, :],
                                 start=(m == 0), stop=(m == M - 1))

            def emit_y_out(w):
                sy = stp.tile([DY, W * BL], F32, tag="sy")
                nc.scalar.copy(sy[:], py_ps[w][:])
                nc.sync.dma_start(
                    y_o[:, bass.ts(w, W), :].rearrange("d t b -> d (t b)"), sy[:])

            def emit_x_mm(w, half):
                if half == 0:
                    px_ps[w] = ps_b.tile([DS, W * BL], F32, tag="psb", name="px")
                h = W * BL // 2
                nc.tensor.matmul(
                    px_ps[w][:, bass.ts(half, h)], sis[:],
                    zh[:, w * W * BL + half * h: w * W * BL + (half + 1) * h],
                    start=True, stop=True)

            def emit_x_out(w):
                sx = stp.tile([DS, W * BL], F32, tag="sx")
                nc.scalar.copy(sx[:], px_ps[w][:])
                nc.sync.dma_start(
                    x_o[:, bass.ts(w, W), :].rearrange("d t b -> d (t b)"), sx[:])

            cw_tiles = {}
            # prologue: window 0 inputs built serially
            emit_pw_dma(0)
            for m in range(M):
                emit_cbuild(0, m)
            emit_cw_copy(0)
            emit_pw_dma(1)

            ps_prev = None
            for w in range(NW):
                ztw = ztwp.tile([DS, W * BL * M], BF16, tag="ztw")
                ztw_tiles[w] = ztw
                pw = pw_tiles[w]
                cw = cw_tiles[w]
                for tr in range(W):
                    t = w * W + tr
                    # ztilde_t
                    zslice = ztw[:, bass.ts(tr, BL * M)]
                    if t == 0:
                        nc.vector.memset(zslice, 0.0)
                    else:
                        nc.vector.tensor_tensor(
                            zslice.rearrange("p (b m) -> p b m", m=M),
                            ps_prev[:][:, :, None].broadcast_to([DS, BL, M]),
                            pw[:, bass.ts(tr, BL * M)].rearrange(
                                "p (b m) -> p b m", m=M),
                            ALU.mult)
                    # recurrence matmuls
                    if t < T - 1:
                        ps = ps_s.tile([DS, BL], F32, tag="pss")
                        nc.tensor.matmul(ps[:], ids[:], cw[:, bass.ts(tr, BL)],
                                         start=True, stop=False)
                        zt_m = zslice.rearrange("p (b m) -> p m b", m=M)
                        for m in range(M):
                            nc.tensor.matmul(ps[:], k11s[:, m, :], zt_m[:, m, :],
                                             start=False, stop=(m == M - 1))
                        nc.scalar.copy(zh[:, bass.ts(t + 1, BL)], ps[:])
                        ps_prev = ps
                    # interleaved window jobs
                    if tr < M and w + 1 < NW:
                        emit_cbuild(w + 1, tr)
                    elif tr == M and w + 1 < NW:
                        emit_cw_copy(w + 1)
                    elif 9 <= tr < 9 + M and w >= 1:
                        emit_y_mm(w - 1, tr - 9)
                    elif tr == 17 and w >= 1:
                        emit_y_out(w - 1)
                    elif tr in (18, 19) and w >= 1:
                        emit_x_mm(w - 1, tr - 18)
                    elif tr == 20 and w >= 1:
                        emit_x_out(w - 1)
                    elif tr == 21 and w + 2 < NW:
                        emit_pw_dma(w + 2)

            # epilogue: last window's y and x
            for m in range(M):
                emit_y_mm(NW - 1, m)
            emit_y_out(NW - 1)
            emit_x_mm(NW - 1, 0)
            emit_x_mm(NW - 1, 1)
            emit_x_out(NW - 1)
    return nc


_CACHED = {}


def _get_program():
    if "nc" not in _CACHED:
        nc = bacc.Bacc()
        build_program(nc)
        nc.finalize()  # Bacc.compile(): reg alloc + sync-wait splitting
        _CACHED["nc"] = nc
    return _CACHED["nc"]


def kernel(u, K_raw, log_gamma, S_raw, gw1, gb1, gw2, gb2):
    p = prepare_params(K_raw, log_gamma, S_raw, gw1, gb1, gw2, gb2)
    nc = _get_program()
    in_maps = make_in_maps(u, p)
    res = run_bass_kernel_spmd(nc, in_maps, list(range(NCORES)))
    y = np.empty((B, T, DY), dtype=np.float32)
    x = np.empty((B, T, DS), dtype=np.float32)
    for i in range(NCORES):
        y[i * BL:(i + 1) * BL] = res.results[i]["y_o"].transpose(2, 1, 0)
        x[i * BL:(i + 1) * BL] = res.results[i]["x_o"].transpose(2, 1, 0)
    return y, x


# revision 25
# speedup vs baseline: 1.2000x; 1.2000x over previous
"""Trainium2 Bass kernel for the expert-selective time-varying SSM.

Math restructuring (vs the reference scan):
  - Track z_t = S @ x_t instead of x_t: the triangular solve disappears
    (z at step t+1 equals the previous step's z_next), x_t = S^-1 z_t is
    recovered by one batched matmul at the end.
  - K22 is zeroed before normalization, so y_t = sum_m pi_m K21_m z_t
    depends only on z_t -> computed outside the scan in batched matmuls.
  - The gate (pi) and the input drive c_t = sum_m pi_m K12_m (g u_t)
    depend only on u -> precomputed in parallel.
  - The only sequential work left: z_{t+1} = sum_m K11_m (pi_m * z_t) + c_t.

Sharding: data-parallel over B across 8 cores (B_loc = 8 per core);
small params replicated. No collectives needed.
"""

import os
import sys

for _p in ("/opt/trn_rl_repo", "/root/.axon_site/_ro/trn_rl_repo"):
    if os.path.isdir(_p) and _p not in sys.path:
        sys.path.insert(0, _p)

import numpy as np
import ml_dtypes

import concourse.bass as bass
import concourse.bacc as bacc
import concourse.tile as tile
from concourse import mybir
from concourse.bass_utils import run_bass_kernel_spmd

F32 = mybir.dt.float32
BF16 = mybir.dt.bfloat16
AF = mybir.ActivationFunctionType
ALU = mybir.AluOpType

B, T, DS, DU, DY, M, GH = 64, 1024, 128, 64, 64, 8, 64
S_DIAG_EPS = 1e-3
NCORES = 8
BL = B // NCORES          # batch per core = 8
W = 64                    # scan window (timesteps)
GELU_MODE = "hw"          # "hw": ACT Gelu; "sim": sigmoid approx (CoreSim dev)
NW = T // W               # number of windows
NTB = T * BL              # (t, b) pairs per core = 8192
CHUNK = 512               # free-dim chunk for phase matmuls
NCHUNK = NTB // CHUNK     # 16


def _np_bf16(x):
    return np.ascontiguousarray(x).astype(ml_dtypes.bfloat16)


def prepare_params(K_raw, log_gamma, S_raw, gw1, gb1, gw2, gb2):
    """Host-side parameter prep (small, O(params) only)."""
    K = np.array(K_raw, dtype=np.float64).copy()
    K[:, DS:DS + DY, DS:DS + DU] = 0.0
    norms = np.array([np.linalg.norm(K[m], 2) for m in range(M)])
    scale = np.maximum(norms, 1.0)
    K = K / scale[:, None, None]

    g = float(np.exp(np.float64(log_gamma)))
    K12 = K[:, :DS, DS:DS + DU] * g           # (M, DS, DU), gamma folded in
    K21 = K[:, DS:DS + DY, :DS]               # (M, DY, DS)

    S = np.tril(np.array(S_raw, dtype=np.float64))
    d = np.diagonal(S).copy()
    sp = np.where(d > 30, d, np.log1p(np.exp(np.minimum(d, 30.0)))) + S_DIAG_EPS
    np.fill_diagonal(S, sp)
    Sinv = np.linalg.inv(S)

    # State-space change to x-coordinates (x = Sinv z): the scan tracks
    # xtil directly, the state output is sum_m ztw (softmax sums to 1),
    # and S / Sinv fold into the weights exactly (fp64 host-side).
    K11 = Sinv @ K[:, :DS, :DS] @ S           # (M, DS, DS)
    K12 = Sinv @ K12                          # (M, DS, DU)
    K21 = K21 @ S                             # (M, DY, DS)

    p = {}
    # lhsT layouts: stationary operand of matmul(out, lhsT, rhs) is [K, M_out]
    p["k11t"] = _np_bf16(K11.transpose(2, 0, 1))          # (DS_in, M, DS_out)
    p["k21t"] = _np_bf16(K21.transpose(2, 0, 1))          # (DS, M, DY)
    p["k12t"] = _np_bf16(K12.transpose(2, 0, 1))          # (DU, M, DS)
    p["gw1t"] = _np_bf16(np.array(gw1, np.float64).T)     # (DU, GH)
    gw2a = np.concatenate([np.array(gw2, np.float64).T,
                           np.array(gb2, np.float64)[None, :]], axis=0)
    p["gw2a"] = _np_bf16(gw2a)                            # (GH+1, M)
    p["gb1c"] = np.array(gb1, np.float64).reshape(GH, 1).astype(np.float32)
    p["idbf"] = _np_bf16(np.eye(DS))                      # c-inject + x-sum
    return p


def make_in_maps(u, p):
    u = np.asarray(u, dtype=np.float32)
    in_maps = []
    for i in range(NCORES):
        m = dict(p)
        ut = np.ascontiguousarray(u[i * BL:(i + 1) * BL].transpose(2, 1, 0))
        m["ut"] = ut                       # (DU, T, BL) f32
        m["utb"] = _np_bf16(ut)
        in_maps.append(m)
    return in_maps


def build_program(nc):
    """Emit the SPMD per-core program. Returns nothing; tensors are
    declared on nc by name."""
    ut = nc.declare_dram_parameter("ut", [DU, T, BL], F32, isOutput=False)
    utb_in = nc.declare_dram_parameter("utb", [DU, T, BL], BF16, isOutput=False)
    idbf = nc.declare_dram_parameter("idbf", [DS, DS], BF16, isOutput=False)
    k11t = nc.declare_dram_parameter("k11t", [DS, M, DS], BF16, isOutput=False)
    k21t = nc.declare_dram_parameter("k21t", [DS, M, DY], BF16, isOutput=False)
    k12t = nc.declare_dram_parameter("k12t", [DU, M, DS], BF16, isOutput=False)
    gw1t = nc.declare_dram_parameter("gw1t", [DU, GH], BF16, isOutput=False)
    gw2a = nc.declare_dram_parameter("gw2a", [GH + 1, M], BF16, isOutput=False)
    gb1c = nc.declare_dram_parameter("gb1c", [GH, 1], F32, isOutput=False)

    y_o = nc.declare_dram_parameter("y_o", [DY, T, BL], F32, isOutput=True)
    x_o = nc.declare_dram_parameter("x_o", [DS, T, BL], F32, isOutput=True)

    pid = nc.dram_tensor("pid", [T, BL, M], BF16)  # pi scratch, (t, b, m)

    with tile.TileContext(nc) as tc:
        with (
            tc.tile_pool(name="wts", bufs=1) as wts,
            tc.tile_pool(name="big", bufs=1) as big,
            tc.tile_pool(name="ph1", bufs=2) as ph1,
            tc.tile_pool(name="winp", bufs=3) as winp,
            tc.tile_pool(name="ztwp", bufs=2) as ztwp,
            tc.tile_pool(name="ub", bufs=2) as ubp,
            tc.tile_pool(name="cw", bufs=2) as cwp,
            tc.tile_pool(name="st", bufs=2) as stp,
            tc.tile_pool(name="ps_s", bufs=3, space="PSUM") as ps_s,
            tc.tile_pool(name="ps_b", bufs=4, space="PSUM") as ps_b,
        ):
            # ---- weight preload ----
            k11s = wts.tile([DS, M, DS], BF16)
            k21s = wts.tile([DS, M, DY], BF16)
            k12s = wts.tile([DU, M, DS], BF16)
            gw1s = wts.tile([DU, GH], BF16)
            gw2s = wts.tile([GH + 1, M], BF16)
            gb1s = wts.tile([GH, 1], F32)
            ids = wts.tile([DS, DS], BF16)
            nc.sync.dma_start(k11s[:], k11t[:])
            nc.sync.dma_start(k21s[:], k21t[:])
            nc.sync.dma_start(k12s[:], k12t[:])
            nc.sync.dma_start(gw1s[:], gw1t[:])
            nc.sync.dma_start(gw2s[:], gw2a[:])
            nc.sync.dma_start(gb1s[:], gb1c[:])
            nc.sync.dma_start(ids[:], idbf[:])

            # ---- persistent big tiles ----
            utf = big.tile([DU, NTB], F32)      # u^T, free = t*BL + b
            utb = big.tile([DU, NTB], BF16)
            nc.sync.dma_start(utf[:], ut[:].rearrange("d t b -> d (t b)"))
            nc.sync.dma_start(utb[:], utb_in[:].rearrange("d t b -> d (t b)"))

            # ---- phase 1: gate -> pi -> DRAM (t, b, m) ----
            hh = big.tile([GH + 1, NTB], BF16)
            nc.vector.memset(hh[GH:GH + 1, :], 1.0)
            for i in range(NCHUNK):
                ph = ps_b.tile([GH, CHUNK], F32, tag="psb")
                nc.tensor.matmul(ph[:], gw1s[:], utb[:, bass.ts(i, CHUNK)],
                                 start=True, stop=True)
                if GELU_MODE == "hw":
                    nc.scalar.activation(hh[:GH, bass.ts(i, CHUNK)], ph[:],
                                         AF.Gelu, bias=gb1s[:], scale=1.0)
                else:
                    pre = ph1.tile([GH, CHUNK], F32, tag="gpre")
                    sg = ph1.tile([GH, CHUNK], F32, tag="gsg")
                    nc.scalar.activation(pre[:], ph[:], AF.Identity,
                                         bias=gb1s[:], scale=1.0)
                    nc.scalar.activation(sg[:], pre[:], AF.Sigmoid, scale=1.702)
                    nc.vector.tensor_mul(hh[:GH, bass.ts(i, CHUNK)],
                                         pre[:], sg[:])

            pl = ps_b.tile([128, NTB // 128 * M], F32, tag="psb")  # (128, 512)
            for c in range(NTB // 128):
                nc.tensor.matmul(pl[:, bass.ts(c, M)], hh[:, bass.ts(c, 128)],
                                 gw2s[:], start=True, stop=True)
            nbt = NTB // 128  # 64
            ex = ph1.tile([128, nbt * M], F32, tag="ex")
            mx = ph1.tile([128, nbt], F32, tag="mx")
            sm = ph1.tile([128, nbt], F32, tag="mx")
            pib = ph1.tile([128, nbt * M], BF16, tag="pib")
            pl3 = pl[:].rearrange("p (c m) -> p c m", m=M)
            nc.vector.tensor_reduce(mx[:], pl3, mybir.AxisListType.X, ALU.max)
            nc.vector.tensor_tensor(ex[:].rearrange("p (c m) -> p c m", m=M),
                                    pl3,
                                    mx[:, :, None].broadcast_to([128, nbt, M]),
                                    ALU.subtract)
            nc.scalar.activation(ex[:], ex[:], AF.Exp)
            nc.vector.tensor_reduce(sm[:], ex[:].rearrange("p (c m) -> p c m", m=M),
                                    mybir.AxisListType.X, ALU.add)
            nc.vector.reciprocal(sm[:], sm[:])
            nc.vector.tensor_tensor(pib[:].rearrange("p (c m) -> p c m", m=M),
                                    ex[:].rearrange("p (c m) -> p c m", m=M),
                                    sm[:, :, None].broadcast_to([128, nbt, M]),
                                    ALU.mult)
            # scatter pi -> DRAM (t, b, m); partition p=q*BL+b maps to
            # t = c*16 + q (bt = c*128 + p, bt = t*BL + b)
            NQ = 128 // BL
            for q in range(NQ):
                src = pib[q * BL:(q + 1) * BL, :].rearrange(
                    "b (c m) -> b c m", m=M)
                dst = pid[:].rearrange("(c q) b m -> q b c m", q=NQ)[q]
                nc.sync.dma_start(dst, src)

            # ---- phase 2: software-pipelined windowed scan ----
            # Per scan step t (window w, tr=t%W):
            #   PE : Id@c_t (start) + 8 accumulating K11 matmuls -> ps = x_{t+1}
            #   DVE: xtil_{t+1} = ps (bcast over m) * P_{t+1}
            # State history never materializes: x_t = sum_m xtil_m(t)
            # (softmax sums to 1), summed per window with Id matmuls.
            # Window jobs (c-build for w+1, y/x for w-1, P prefetch w+2) are
            # interleaved one per scan step to fill engine idle time.
            pw_tiles = {}
            ztw_tiles = {}
            pc_ps = {}
            py_ps = {}
            px_ps = {}

            def emit_pw_dma(w):
                pw = winp.tile([128, W * BL * M], BF16, tag="pw")
                nc.sync.dma_start(
                    pw[:],
                    pid[bass.ts(w, W), :, :].rearrange("t b m -> (t b m)")
                    .partition_broadcast(128)
                    .rearrange("p f -> p f"))
                pw_tiles[w] = pw

            def emit_cbuild(w, m):
                # one expert of the c-window accumulation
                if m == 0:
                    pc_ps[w] = ps_b.tile([DS, W * BL], F32, tag="psb", name="pc")
                pw = pw_tiles[w]
                ub = ubp.tile([DU, W * BL], BF16, tag="ub")
                nc.vector.tensor_tensor(
                    ub[:].rearrange("p (t b) -> p t b", b=BL),
                    utf[:, bass.ts(w, W * BL)].rearrange("p (t b) -> p t b", b=BL),
                    pw[:DU, :].rearrange("p (t b m) -> p t b m", b=BL, m=M)[:, :, :, m],
                    ALU.mult)
                nc.tensor.matmul(pc_ps[w][:], k12s[:, m, :], ub[:],
                                 start=(m == 0), stop=(m == M - 1))

            def emit_cw_copy(w):
                cw = cwp.tile([DS, W * BL], BF16, tag="cw")
                nc.scalar.copy(cw[:], pc_ps[w][:])
                cw_tiles[w] = cw

            def emit_y_mm(w, m):
                if m == 0:
                    py_ps[w] = ps_b.tile([DY, W * BL], F32, tag="psb", name="py")
                ztw4 = ztw_tiles[w][:].rearrange(
                    "p (t b m) -> p m t b", b=BL, m=M)
                nc.tensor.matmul(py_ps[w][:], k21s[:, m, :], ztw4[:, m, :, :],
                                 start=(m == 0), stop=(m == M - 1))

            def emit_y_out(w):
                sy = stp.tile([DY, W * BL], F32, tag="sy")
                nc.scalar.copy(sy[:], py_ps[w][:])
                nc.sync.dma_start(
                    y_o[:, bass.ts(w, W), :].rearrange("d t b -> d (t b)"), sy[:])

            def emit_x_mm(w, m):
                # x_t = sum_m xtil_m(t): accumulate ztw m-slices via Id
                if m == 0:
                    px_ps[w] = ps_b.tile([DS, W * BL], F32, tag="psb", name="px")
                ztw4 = ztw_tiles[w][:].rearrange(
                    "p (t b m) -> p m t b", b=BL, m=M)
                nc.tensor.matmul(px_ps[w][:], ids[:], ztw4[:, m, :, :],
                                 start=(m == 0), stop=(m == M - 1))

            def emit_x_out(w):
                sx = stp.tile([DS, W * BL], F32, tag="sx")
                nc.scalar.copy(sx[:], px_ps[w][:])
                nc.sync.dma_start(
                    x_o[:, bass.ts(w, W), :].rearrange("d t b -> d (t b)"), sx[:])

            cw_tiles = {}
            # prologue: window 0 inputs built serially
            emit_pw_dma(0)
            for m in range(M):
                emit_cbuild(0, m)
            emit_cw_copy(0)
            emit_pw_dma(1)

            ps_prev = None
            for w in range(NW):
                ztw = ztwp.tile([DS, W * BL * M], BF16, tag="ztw")
                ztw_tiles[w] = ztw
                pw = pw_tiles[w]
                cw = cw_tiles[w]
                for tr in range(W):
                    t = w * W + tr
                    # ztilde_t
                    zslice = ztw[:, bass.ts(tr, BL * M)]
                    if t == 0:
                        nc.vector.memset(zslice, 0.0)
                    else:
                        nc.vector.tensor_tensor(
                            zslice.rearrange("p (b m) -> p b m", m=M),
                            ps_prev[:][:, :, None].broadcast_to([DS, BL, M]),
                            pw[:, bass.ts(tr, BL * M)].rearrange(
                                "p (b m) -> p b m", m=M),
                            ALU.mult)
                    # recurrence matmuls
                    if t < T - 1:
                        ps = ps_s.tile([DS, BL], F32, tag="pss")
                        nc.tensor.matmul(ps[:], ids[:], cw[:, bass.ts(tr, BL)],
                                         start=True, stop=False)
                        zt_m = zslice.rearrange("p (b m) -> p m b", m=M)
                        for m in range(M):
                            nc.tensor.matmul(ps[:], k11s[:, m, :], zt_m[:, m, :],
                                             start=False, stop=(m == M - 1))
                        ps_prev = ps
                    # interleaved window jobs
                    if tr < M and w + 1 < NW:
                        emit_cbuild(w + 1, tr)
                    elif tr == M and w + 1 < NW:
                        emit_cw_copy(w + 1)
                    elif 9 <= tr < 9 + M and w >= 1:
                        emit_y_mm(w - 1, tr - 9)
                    elif tr == 17 and w >= 1:
                        emit_y_out(w - 1)
                    elif 18 <= tr < 18 + M and w >= 1:
                        emit_x_mm(w - 1, tr - 18)
                    elif tr == 26 and w >= 1:
                        emit_x_out(w - 1)
                    elif tr == 27 and w + 2 < NW:
                        emit_pw_dma(w + 2)

            # epilogue: last window's y and x
            for m in range(M):
                emit_y_mm(NW - 1, m)
            emit_y_out(NW - 1)
            for m in range(M):
                emit_x_mm(NW - 1, m)
            emit_x_out(NW - 1)
    return nc


_CACHED = {}


def _get_program():
    if "nc" not in _CACHED:
        nc = bacc.Bacc()
        build_program(nc)
        nc.finalize()  # Bacc.compile(): reg alloc + sync-wait splitting
        _CACHED["nc"] = nc
    return _CACHED["nc"]


def kernel(u, K_raw, log_gamma, S_raw, gw1, gb1, gw2, gb2):
    p = prepare_params(K_raw, log_gamma, S_raw, gw1, gb1, gw2, gb2)
    nc = _get_program()
    in_maps = make_in_maps(u, p)
    res = run_bass_kernel_spmd(nc, in_maps, list(range(NCORES)))
    y = np.empty((B, T, DY), dtype=np.float32)
    x = np.empty((B, T, DS), dtype=np.float32)
    for i in range(NCORES):
        y[i * BL:(i + 1) * BL] = res.results[i]["y_o"].transpose(2, 1, 0)
        x[i * BL:(i + 1) * BL] = res.results[i]["x_o"].transpose(2, 1, 0)
    return y, x


# revision 42
# speedup vs baseline: 1.3245x; 1.1037x over previous
"""Trainium2 Bass kernel for the expert-selective time-varying SSM.

Math restructuring (vs the reference scan):
  - Track z_t = S @ x_t instead of x_t: the triangular solve disappears
    (z at step t+1 equals the previous step's z_next), x_t = S^-1 z_t is
    recovered by one batched matmul at the end.
  - K22 is zeroed before normalization, so y_t = sum_m pi_m K21_m z_t
    depends only on z_t -> computed outside the scan in batched matmuls.
  - The gate (pi) and the input drive c_t = sum_m pi_m K12_m (g u_t)
    depend only on u -> precomputed in parallel.
  - The only sequential work left: z_{t+1} = sum_m K11_m (pi_m * z_t) + c_t.

Sharding: data-parallel over B across 8 cores (B_loc = 8 per core);
small params replicated. No collectives needed.
"""

import os
import sys

for _p in ("/opt/trn_rl_repo", "/root/.axon_site/_ro/trn_rl_repo"):
    if os.path.isdir(_p) and _p not in sys.path:
        sys.path.insert(0, _p)

import numpy as np
import ml_dtypes

import concourse.bass as bass
import concourse.bacc as bacc
import concourse.tile as tile
from concourse import mybir
from concourse.bass_utils import run_bass_kernel_spmd

F32 = mybir.dt.float32
BF16 = mybir.dt.bfloat16
AF = mybir.ActivationFunctionType
ALU = mybir.AluOpType

B, T, DS, DU, DY, M, GH = 64, 1024, 128, 64, 64, 8, 64
S_DIAG_EPS = 1e-3
NCORES = 8
BL = B // NCORES          # batch per core = 8
W = 64                    # scan window (timesteps)
GELU_MODE = "hw"          # "hw": ACT Gelu; "sim": sigmoid approx (CoreSim dev)
NW = T // W               # number of windows
NTB = T * BL              # (t, b) pairs per core = 8192
CHUNK = 512               # free-dim chunk for phase matmuls
NCHUNK = NTB // CHUNK     # 16


def _np_bf16(x):
    return np.ascontiguousarray(x).astype(ml_dtypes.bfloat16)


def prepare_params(K_raw, log_gamma, S_raw, gw1, gb1, gw2, gb2):
    """Host-side parameter prep (small, O(params) only)."""
    K = np.array(K_raw, dtype=np.float64).copy()
    K[:, DS:DS + DY, DS:DS + DU] = 0.0
    norms = np.array([np.linalg.norm(K[m], 2) for m in range(M)])
    scale = np.maximum(norms, 1.0)
    K = K / scale[:, None, None]

    g = float(np.exp(np.float64(log_gamma)))
    K12 = K[:, :DS, DS:DS + DU] * g           # (M, DS, DU), gamma folded in
    K21 = K[:, DS:DS + DY, :DS]               # (M, DY, DS)

    S = np.tril(np.array(S_raw, dtype=np.float64))
    d = np.diagonal(S).copy()
    sp = np.where(d > 30, d, np.log1p(np.exp(np.minimum(d, 30.0)))) + S_DIAG_EPS
    np.fill_diagonal(S, sp)
    Sinv = np.linalg.inv(S)

    # State-space change to x-coordinates (x = Sinv z): the scan tracks
    # xtil directly, the state output is sum_m ztw (softmax sums to 1),
    # and S / Sinv fold into the weights exactly (fp64 host-side).
    K11 = Sinv @ K[:, :DS, :DS] @ S           # (M, DS, DS)
    K12 = Sinv @ K12                          # (M, DS, DU)
    K21 = K21 @ S                             # (M, DY, DS)

    p = {}
    # lhsT layouts: stationary operand of matmul(out, lhsT, rhs) is [K, M_out]
    p["k11t"] = _np_bf16(K11.transpose(2, 0, 1))          # (DS_in, M, DS_out)
    p["k21t"] = _np_bf16(K21.transpose(2, 0, 1))          # (DS, M, DY)
    p["k12t"] = _np_bf16(K12.transpose(2, 0, 1))          # (DU, M, DS)
    p["gw1t"] = _np_bf16(np.array(gw1, np.float64).T)     # (DU, GH)
    gw2a = np.concatenate([np.array(gw2, np.float64).T,
                           np.array(gb2, np.float64)[None, :]], axis=0)
    p["gw2a"] = _np_bf16(gw2a)                            # (GH+1, M)
    p["gb1c"] = np.array(gb1, np.float64).reshape(GH, 1).astype(np.float32)
    p["idbf"] = _np_bf16(np.eye(DS))                      # c-inject + x-sum
    return p


def make_in_maps(u, p):
    u = np.asarray(u, dtype=np.float32)
    in_maps = []
    for i in range(NCORES):
        m = dict(p)
        ut = np.ascontiguousarray(u[i * BL:(i + 1) * BL].transpose(2, 1, 0))
        m["ut"] = ut                       # (DU, T, BL) f32
        m["utb"] = _np_bf16(ut)
        in_maps.append(m)
    return in_maps


def build_program(nc):
    """Emit the SPMD per-core program. Returns nothing; tensors are
    declared on nc by name."""
    ut = nc.declare_dram_parameter("ut", [DU, T, BL], F32, isOutput=False)
    utb_in = nc.declare_dram_parameter("utb", [DU, T, BL], BF16, isOutput=False)
    idbf = nc.declare_dram_parameter("idbf", [DS, DS], BF16, isOutput=False)
    k11t = nc.declare_dram_parameter("k11t", [DS, M, DS], BF16, isOutput=False)
    k21t = nc.declare_dram_parameter("k21t", [DS, M, DY], BF16, isOutput=False)
    k12t = nc.declare_dram_parameter("k12t", [DU, M, DS], BF16, isOutput=False)
    gw1t = nc.declare_dram_parameter("gw1t", [DU, GH], BF16, isOutput=False)
    gw2a = nc.declare_dram_parameter("gw2a", [GH + 1, M], BF16, isOutput=False)
    gb1c = nc.declare_dram_parameter("gb1c", [GH, 1], F32, isOutput=False)

    y_o = nc.declare_dram_parameter("y_o", [DY, T, BL], F32, isOutput=True)
    x_o = nc.declare_dram_parameter("x_o", [DS, T, BL], F32, isOutput=True)

    pid = nc.dram_tensor("pid", [T, BL, M], BF16)  # pi scratch, (t, b, m)

    with tile.TileContext(nc) as tc:
        with (
            tc.tile_pool(name="wts", bufs=1) as wts,
            tc.tile_pool(name="big", bufs=1) as big,
            tc.tile_pool(name="ph1", bufs=2) as ph1,
            tc.tile_pool(name="winp", bufs=3) as winp,
            tc.tile_pool(name="ztwp", bufs=2) as ztwp,
            tc.tile_pool(name="ub", bufs=2) as ubp,
            tc.tile_pool(name="cw", bufs=2) as cwp,
            tc.tile_pool(name="st", bufs=2) as stp,
            tc.tile_pool(name="ps_s", bufs=3, space="PSUM") as ps_s,
            tc.tile_pool(name="ps_b", bufs=4, space="PSUM") as ps_b,
        ):
            # ---- weight preload ----
            k11s = wts.tile([DS, M, DS], BF16)
            k21s = wts.tile([DS, M, DY], BF16)
            k12s = wts.tile([DU, M, DS], BF16)
            gw1s = wts.tile([DU, GH], BF16)
            gw2s = wts.tile([GH + 1, M], BF16)
            gb1s = wts.tile([GH, 1], F32)
            ids = wts.tile([DS, DS], BF16)
            nc.sync.dma_start(k11s[:], k11t[:])
            nc.sync.dma_start(k21s[:], k21t[:])
            nc.sync.dma_start(k12s[:], k12t[:])
            nc.sync.dma_start(gw1s[:], gw1t[:])
            nc.sync.dma_start(gw2s[:], gw2a[:])
            nc.sync.dma_start(gb1s[:], gb1c[:])
            nc.sync.dma_start(ids[:], idbf[:])

            # ---- persistent big tiles ----
            utf = big.tile([DU, NTB], F32)      # u^T, free = t*BL + b
            utb = big.tile([DU, NTB], BF16)
            nc.sync.dma_start(utf[:], ut[:].rearrange("d t b -> d (t b)"))
            nc.sync.dma_start(utb[:], utb_in[:].rearrange("d t b -> d (t b)"))

            # ---- phase 1: gate -> pi -> DRAM (t, b, m) ----
            hh = big.tile([GH + 1, NTB], BF16)
            nc.vector.memset(hh[GH:GH + 1, :], 1.0)
            for i in range(NCHUNK):
                ph = ps_b.tile([GH, CHUNK], F32, tag="psb")
                nc.tensor.matmul(ph[:], gw1s[:], utb[:, bass.ts(i, CHUNK)],
                                 start=True, stop=True)
                if GELU_MODE == "hw":
                    nc.scalar.activation(hh[:GH, bass.ts(i, CHUNK)], ph[:],
                                         AF.Gelu, bias=gb1s[:], scale=1.0)
                else:
                    pre = ph1.tile([GH, CHUNK], F32, tag="gpre")
                    sg = ph1.tile([GH, CHUNK], F32, tag="gsg")
                    nc.scalar.activation(pre[:], ph[:], AF.Identity,
                                         bias=gb1s[:], scale=1.0)
                    nc.scalar.activation(sg[:], pre[:], AF.Sigmoid, scale=1.702)
                    nc.vector.tensor_mul(hh[:GH, bass.ts(i, CHUNK)],
                                         pre[:], sg[:])

            pl = ps_b.tile([128, NTB // 128 * M], F32, tag="psb")  # (128, 512)
            for c in range(NTB // 128):
                nc.tensor.matmul(pl[:, bass.ts(c, M)], hh[:, bass.ts(c, 128)],
                                 gw2s[:], start=True, stop=True)
            nbt = NTB // 128  # 64
            ex = ph1.tile([128, nbt * M], F32, tag="ex")
            mx = ph1.tile([128, nbt], F32, tag="mx")
            sm = ph1.tile([128, nbt], F32, tag="mx")
            pib = ph1.tile([128, nbt * M], BF16, tag="pib")
            pl3 = pl[:].rearrange("p (c m) -> p c m", m=M)
            nc.vector.tensor_reduce(mx[:], pl3, mybir.AxisListType.X, ALU.max)
            nc.vector.tensor_tensor(ex[:].rearrange("p (c m) -> p c m", m=M),
                                    pl3,
                                    mx[:, :, None].broadcast_to([128, nbt, M]),
                                    ALU.subtract)
            nc.scalar.activation(ex[:], ex[:], AF.Exp)
            nc.vector.tensor_reduce(sm[:], ex[:].rearrange("p (c m) -> p c m", m=M),
                                    mybir.AxisListType.X, ALU.add)
            nc.vector.reciprocal(sm[:], sm[:])
            nc.vector.tensor_tensor(pib[:].rearrange("p (c m) -> p c m", m=M),
                                    ex[:].rearrange("p (c m) -> p c m", m=M),
                                    sm[:, :, None].broadcast_to([128, nbt, M]),
                                    ALU.mult)
            # scatter pi -> DRAM (t, m, b); partition p=q*BL+b maps to
            # t = c*16 + q (bt = c*128 + p, bt = t*BL + b)
            NQ = 128 // BL
            for q in range(NQ):
                src = pib[q * BL:(q + 1) * BL, :].rearrange(
                    "b (c m) -> b c m", m=M)
                dst = pid[:].rearrange("(c q) b m -> q b c m", q=NQ)[q]
                nc.sync.dma_start(dst, src)

            # ---- phase 2: software-pipelined windowed scan ----
            # Per scan step t (window w, tr=t%W):
            #   PE : Id@c_t (start) + 8 accumulating K11 matmuls -> ps = x_{t+1}
            #   DVE: xtil_{t+1} = ps (bcast over m) * P_{t+1}
            # State history never materializes: x_t = sum_m xtil_m(t)
            # (softmax sums to 1), summed per window with Id matmuls.
            # Window jobs (c-build for w+1, y/x for w-1, P prefetch w+2) are
            # interleaved one per scan step to fill engine idle time.
            pw_tiles = {}
            ztw_tiles = {}
            pc_ps = {}
            py_ps = {}
            px_ps = {}

            def emit_pw_dma(w):
                # 4 time-chunk DMAs: parallel queues + earlier availability
                pw = winp.tile([128, W * BL * M], BF16, tag="pw")
                ck = W // 4
                for k in range(4):
                    nc.sync.dma_start(
                        pw[:, k * ck * BL * M:(k + 1) * ck * BL * M],
                        pid[w * W + k * ck:w * W + (k + 1) * ck, :, :]
                        .rearrange("t b m -> (t b m)")
                        .partition_broadcast(128)
                        .rearrange("p f -> p f"))
                pw_tiles[w] = pw

            def emit_cbuild(w, m):
                # one expert of the c-window accumulation
                if m == 0:
                    pc_ps[w] = ps_b.tile([DS, W * BL], F32, tag="psb", name="pc")
                pw = pw_tiles[w]
                ub = ubp.tile([DU, W * BL], BF16, tag="ub")
                nc.vector.tensor_tensor(
                    ub[:].rearrange("p (t b) -> p t b", b=BL),
                    utf[:, bass.ts(w, W * BL)].rearrange("p (t b) -> p t b", b=BL),
                    pw[:DU, :].rearrange("p (t b m) -> p t b m", b=BL, m=M)[:, :, :, m],
                    ALU.mult)
                nc.tensor.matmul(pc_ps[w][:], k12s[:, m, :], ub[:],
                                 start=(m == 0), stop=(m == M - 1))

            def emit_cw_copy(w):
                cw = cwp.tile([DS, W * BL], BF16, tag="cw")
                nc.scalar.copy(cw[:], pc_ps[w][:])
                cw_tiles[w] = cw

            def emit_y_mm(w, m):
                if m == 0:
                    py_ps[w] = ps_b.tile([DY, W * BL], F32, tag="psb", name="py")
                ztw4 = ztw_tiles[w][:].rearrange(
                    "p (t m b) -> p m t b", b=BL, m=M)
                nc.tensor.matmul(py_ps[w][:], k21s[:, m, :], ztw4[:, m, :, :],
                                 start=(m == 0), stop=(m == M - 1))

            def emit_y_out(w):
                sy = stp.tile([DY, W * BL], F32, tag="sy")
                nc.scalar.copy(sy[:], py_ps[w][:])
                nc.sync.dma_start(
                    y_o[:, bass.ts(w, W), :].rearrange("d t b -> d (t b)"), sy[:])

            def emit_x_mm(w, m):
                # x_t = sum_m xtil_m(t): accumulate ztw m-slices via Id
                if m == 0:
                    px_ps[w] = ps_b.tile([DS, W * BL], F32, tag="psb", name="px")
                ztw4 = ztw_tiles[w][:].rearrange(
                    "p (t m b) -> p m t b", b=BL, m=M)
                nc.tensor.matmul(px_ps[w][:], ids[:], ztw4[:, m, :, :],
                                 start=(m == 0), stop=(m == M - 1))

            def emit_x_out(w):
                sx = stp.tile([DS, W * BL], F32, tag="sx")
                nc.scalar.copy(sx[:], px_ps[w][:])
                nc.sync.dma_start(
                    x_o[:, bass.ts(w, W), :].rearrange("d t b -> d (t b)"), sx[:])

            cw_tiles = {}
            # prologue: window 0 inputs built serially
            emit_pw_dma(0)
            for m in range(M):
                emit_cbuild(0, m)
            emit_cw_copy(0)
            emit_pw_dma(1)

            ps_prev = None
            for w in range(NW):
                ztw = ztwp.tile([DS, W * BL * M], BF16, tag="ztw")
                ztw_tiles[w] = ztw
                pw = pw_tiles[w]
                cw = cw_tiles[w]
                for tr in range(W):
                    t = w * W + tr
                    # ztilde_t: ztw layout (t, m, b) -> contiguous writes and
                    # contiguous scan-MM reads; y/x read 16B runs
                    zslice = ztw[:, bass.ts(tr, BL * M)]
                    if t == 0:
                        nc.vector.memset(zslice, 0.0)
                    else:
                        nc.vector.tensor_tensor(
                            zslice.rearrange("p (m b) -> p m b", m=M),
                            ps_prev[:][:, None, :].broadcast_to([DS, M, BL]),
                            pw[:, bass.ts(tr, BL * M)].rearrange(
                                "p (b m) -> p m b", m=M),
                            ALU.mult)
                    # recurrence matmuls
                    if t < T - 1:
                        ps = ps_s.tile([DS, BL], F32, tag="pss")
                        nc.tensor.matmul(ps[:], ids[:], cw[:, bass.ts(tr, BL)],
                                         start=True, stop=False)
                        for m in range(M):
                            nc.tensor.matmul(
                                ps[:], k11s[:, m, :],
                                ztw[:, tr * BL * M + m * BL:
                                    tr * BL * M + (m + 1) * BL],
                                start=False, stop=(m == M - 1))
                        ps_prev = ps
                    # interleaved window jobs
                    if tr < M and w + 1 < NW:
                        emit_cbuild(w + 1, tr)
                    elif tr == M and w + 1 < NW:
                        emit_cw_copy(w + 1)
                    elif 9 <= tr < 9 + M and w >= 1:
                        emit_y_mm(w - 1, tr - 9)
                    elif tr == 17 and w >= 1:
                        emit_y_out(w - 1)
                    elif 18 <= tr < 18 + M and w >= 1:
                        emit_x_mm(w - 1, tr - 18)
                    elif tr == 26 and w >= 1:
                        emit_x_out(w - 1)
                    elif tr == 27 and w + 2 < NW:
                        emit_pw_dma(w + 2)

            # epilogue: last window's y and x
            for m in range(M):
                emit_y_mm(NW - 1, m)
            emit_y_out(NW - 1)
            for m in range(M):
                emit_x_mm(NW - 1, m)
            emit_x_out(NW - 1)
    return nc


_CACHED = {}


def _get_program():
    if "nc" not in _CACHED:
        nc = bacc.Bacc()
        build_program(nc)
        nc.finalize()  # Bacc.compile(): reg alloc + sync-wait splitting
        _CACHED["nc"] = nc
    return _CACHED["nc"]


def kernel(u, K_raw, log_gamma, S_raw, gw1, gb1, gw2, gb2):
    p = prepare_params(K_raw, log_gamma, S_raw, gw1, gb1, gw2, gb2)
    nc = _get_program()
    in_maps = make_in_maps(u, p)
    res = run_bass_kernel_spmd(nc, in_maps, list(range(NCORES)))
    y = np.empty((B, T, DY), dtype=np.float32)
    x = np.empty((B, T, DS), dtype=np.float32)
    for i in range(NCORES):
        y[i * BL:(i + 1) * BL] = res.results[i]["y_o"].transpose(2, 1, 0)
        x[i * BL:(i + 1) * BL] = res.results[i]["x_o"].transpose(2, 1, 0)
    return y, x
